# revision 8
# baseline (speedup 1.0000x reference)
"""Trainium2 Bass kernel for CosineSimCodebook eval forward.

Problem (hardcoded): x [8, 4096, 512] f32, embeddings [1, 8192, 512] f32.
Returns (quantize [8,4096,512] f32, embed_ind [8,4096] i32, dist [1,8,4096,8192] f32).

Sharding: data-parallel over batch dim b (8 cores, one batch slice each);
codebook replicated. No collectives needed (eval mode).

Per-core device kernel:
  dist = x_b @ e.T     on PE in float32r (fast fp32 mode, ~3.2x over fp32;
                       measured abs noise ~2e-4 rms per dot product)
  hierarchical argmax  ACT evicts PSUM->SBUF; DVE reduces each row to 256
                       sub-group maxima (32 codes each), then max/max_index
                       pick the top-8 sub-groups + the winning sub-group.

Host side: inputs are pre-transposed (xT [512,4096], eT [512,8192]) so the
contraction dim lands on SBUF partitions with natural-layout DMAs. The host
finishes the argmax inside the winning 32-wide sub-group (using the same
device dist values), derives the exact global top-2 gap from sub-group
maxima, and for tokens whose gap is inside the f32r noise band recomputes
the near-tie candidates in f64 so embed_ind/quantize match exact-fp32
argmax semantics. quantize is a host gather of the codebook rows.
"""

import numpy as np

B, N, D, C, P = 8, 4096, 512, 8192, 128
NT = N // P     # 32 token tiles per core
KC = D // P     # 4 contraction chunks
FD = 512        # matmul free dim (one PSUM bank of f32)
NG = C // FD    # 16 code groups
GW = 32         # sub-group width for hierarchical argmax
NGRP = C // GW  # 256 sub-groups per token

MODE = "f32r"      # "f32r" or "fp32"
GAP_THRESH = 4e-3  # host fix-up threshold on top1-top2 gap (f32r noise ~2e-4 rms)

_cache = {}


def _build():
    import concourse.bacc as bacc
    import concourse.tile as tile
    from concourse import mybir

    f32 = mybir.dt.float32
    u32 = mybir.dt.uint32
    mm_dt = f32 if MODE == "fp32" else mybir.dt.float32r

    nc = bacc.Bacc(None, target_bir_lowering=False, debug=False)

    xT = nc.dram_tensor("xT", [KC, P, N], mm_dt, kind="ExternalInput")
    eT = nc.dram_tensor("eT", [D, C], mm_dt, kind="ExternalInput")
    dist = nc.dram_tensor("dist", [N, C], f32, kind="ExternalOutput")
    vout = nc.dram_tensor("vout", [N, 8], f32, kind="ExternalOutput")
    iout = nc.dram_tensor("iout", [N, 8], u32, kind="ExternalOutput")

    with tile.TileContext(nc) as tc:
        with (
            tc.tile_pool(name="econst", bufs=1) as epool,
            tc.tile_pool(name="xp", bufs=3) as xpool,
            tc.tile_pool(name="dp", bufs=4) as dpool,
            tc.tile_pool(name="mx", bufs=2) as mpool,
            tc.tile_pool(name="ps", bufs=8, space="PSUM") as pspool,
        ):
            # eT as 16 independent column-chunk tiles (one per 512-code matmul
            # group) so the first matmuls only wait on 1MB of codebook, and
            # later chunks stream in behind the PE.
            eT_sbs = []
            for g in range(NG):
                t = epool.tile([P, KC, FD], mm_dt, tag=f"eT{g}")
                for k in range(KC):
                    nc.sync.dma_start(
                        t[:, k, :],
                        eT[k * P:(k + 1) * P, g * FD:(g + 1) * FD],
                    )
                eT_sbs.append(t)

            for m in range(NT):
                ms = slice(m * P, (m + 1) * P)
                xT_sb = xpool.tile([P, KC, P], mm_dt)
                nc.sync.dma_start(
                    xT_sb[:],
                    xT[:, :, ms].rearrange("k p t -> p k t"),
                )

                # two half-row dist buffers per tile for finer recycling
                gmax = mpool.tile([P, NGRP], f32)
                for h in range(2):
                    dist_sb = dpool.tile([P, C // 2], f32, tag="dist_h")
                    for gg in range(NG // 2):
                        g = h * (NG // 2) + gg
                        gs = slice(gg * FD, (gg + 1) * FD)
                        ps = pspool.tile([P, FD], f32)
                        for k in range(KC):
                            nc.tensor.matmul(
                                ps[:],
                                xT_sb[:, k, :],
                                eT_sbs[g][:, k, :],
                                start=(k == 0),
                                stop=(k == KC - 1),
                            )
                        nc.scalar.copy(dist_sb[:, gs], ps[:])
                        # stream out in 1MB chunks of 4 groups
                        if gg % 4 == 3:
                            cs = slice((gg - 3) * FD, (gg + 1) * FD)
                            ds = slice(h * C // 2 + (gg - 3) * FD,
                                       h * C // 2 + (gg + 1) * FD)
                            nc.sync.dma_start(dist[ms, ds], dist_sb[:, cs])
                    nc.vector.tensor_reduce(
                        gmax[:, h * NGRP // 2:(h + 1) * NGRP // 2],
                        dist_sb[:].rearrange("p (g w) -> p g w", w=GW),
                        axis=mybir.AxisListType.X,
                        op=mybir.AluOpType.max,
                    )
                max8 = mpool.tile([P, 8], f32)
                idx8 = mpool.tile([P, 8], u32)
                nc.vector.max(max8[:], gmax[:])
                nc.vector.max_index(idx8[:], max8[:], gmax[:])
                nc.sync.dma_start(vout[ms, :], max8[:])
                nc.sync.dma_start(iout[ms, :], idx8[:])

    nc.compile()
    return nc


def _host_finish(x_flat, e, dist_flat, max8, idx8):
    """Finish the argmax: exact index inside the winning 32-wide sub-group,
    exact global top-2 gap, f64 fix-up of tokens inside the noise band.

    Returns (embed_ind [T] int64, n_fixed)."""
    T = dist_flat.shape[0]
    g1 = idx8[:, 0].astype(np.int64)                       # winning sub-group
    cols = g1[:, None] * GW + np.arange(GW)[None, :]
    slices = dist_flat[np.arange(T)[:, None], cols]        # [T, GW]
    li = np.argmax(slices, axis=1)
    idx = g1 * GW + li
    top1 = slices[np.arange(T), li]
    s2 = np.partition(slices, GW - 2, axis=1)[:, GW - 2]   # 2nd within group
    second = np.maximum(s2, max8[:, 1])                    # exact global 2nd
    flagged = np.nonzero(top1 - second < GAP_THRESH)[0]
    for t in flagged:
        cand = np.nonzero(dist_flat[t] >= top1[t] - 2.0 * GAP_THRESH)[0]
        exact = e[cand].astype(np.float64) @ x_flat[t].astype(np.float64)
        idx[t] = cand[int(np.argmax(exact))]
    return idx, len(flagged)


def run(x, embeddings, trace=False):
    from concourse.bass_utils import run_bass_kernel_spmd

    if "nc" not in _cache:
        _cache["nc"] = _build()
    nc = _cache["nc"]

    x = np.asarray(x, dtype=np.float32)
    e = np.ascontiguousarray(np.asarray(embeddings, dtype=np.float32)[0])  # [C, D]
    eT = np.ascontiguousarray(e.T)  # [D, C]

    in_maps = [
        {
            "xT": np.ascontiguousarray(x[b].T).reshape(KC, P, N),
            "eT": eT,
        }
        for b in range(B)
    ]
    out = run_bass_kernel_spmd(nc, in_maps, list(range(B)), trace=trace)
    res = out.results

    dist = np.stack([res[b]["dist"] for b in range(B)])[None]  # [1,8,N,C]
    embed_ind = np.empty((B, N), dtype=np.int64)
    nfix = 0
    for b in range(B):
        idx, nf = _host_finish(
            x[b], e, dist[0, b], res[b]["vout"], res[b]["iout"]
        )
        embed_ind[b] = idx
        nfix += nf
    run.last_nfix = nfix

    quantize = e[embed_ind]  # [8, N, D]
    return (quantize, embed_ind.astype(np.int32), dist), out


def kernel(x, embeddings):
    (quantize, embed_ind, dist), _ = run(x, embeddings, trace=False)
    return quantize, embed_ind, dist


# revision 12
# speedup vs baseline: 1.0795x; 1.0795x over previous
"""Trainium2 Bass kernel for CosineSimCodebook eval forward.

Problem (hardcoded): x [8, 4096, 512] f32, embeddings [1, 8192, 512] f32.
Returns (quantize [8,4096,512] f32, embed_ind [8,4096] i32, dist [1,8,4096,8192] f32).

Sharding: data-parallel over batch dim b (8 cores, one batch slice each);
codebook replicated. No collectives needed (eval mode).

Per-core device kernel:
  dist = x_b @ e.T     on PE in float32r (fast fp32 mode, ~3.2x over fp32;
                       measured abs noise ~2e-4 rms per dot product)
  hierarchical argmax  ACT evicts PSUM->SBUF; DVE reduces each row to 256
                       sub-group maxima (32 codes each), then max/max_index
                       pick the top-8 sub-groups + the winning sub-group.

Host side: inputs are pre-transposed (xT [512,4096], eT [512,8192]) so the
contraction dim lands on SBUF partitions with natural-layout DMAs. The host
finishes the argmax inside the winning 32-wide sub-group (using the same
device dist values), derives the exact global top-2 gap from sub-group
maxima, and for tokens whose gap is inside the f32r noise band recomputes
the near-tie candidates in f64 so embed_ind/quantize match exact-fp32
argmax semantics. quantize is a host gather of the codebook rows.
"""

import numpy as np

B, N, D, C, P = 8, 4096, 512, 8192, 128
NT = N // P     # 32 token tiles per core
KC = D // P     # 4 contraction chunks
FD = 512        # matmul free dim (one PSUM bank of f32)
NG = C // FD    # 16 code groups
GW = 32         # sub-group width for hierarchical argmax
NGRP = C // GW  # 256 sub-groups per token

MODE = "f32r"      # "f32r" or "fp32"
GAP_THRESH = 4e-3  # host fix-up threshold on top1-top2 gap (f32r noise ~2e-4 rms)

_cache = {}


def _build():
    import concourse.bacc as bacc
    import concourse.tile as tile
    from concourse import mybir

    f32 = mybir.dt.float32
    u32 = mybir.dt.uint32
    mm_dt = f32 if MODE == "fp32" else mybir.dt.float32r

    nc = bacc.Bacc(None, target_bir_lowering=False, debug=False)

    xT = nc.dram_tensor("xT", [KC, P, N], mm_dt, kind="ExternalInput")
    eT = nc.dram_tensor("eT", [D, C], mm_dt, kind="ExternalInput")
    dist = nc.dram_tensor("dist", [N, C], f32, kind="ExternalOutput")
    vout = nc.dram_tensor("vout", [N, 8], f32, kind="ExternalOutput")
    iout = nc.dram_tensor("iout", [N, 8], u32, kind="ExternalOutput")

    with tile.TileContext(nc) as tc:
        with (
            tc.tile_pool(name="econst", bufs=1) as epool,
            tc.tile_pool(name="xp", bufs=2) as xpool,
            tc.tile_pool(name="dp", bufs=4) as dpool,
            tc.tile_pool(name="mx", bufs=1) as mpool,
            tc.tile_pool(name="ps", bufs=8, space="PSUM") as pspool,
        ):
            # eT as 16 independent column-chunk tiles (one per 512-code matmul
            # group) so the first matmuls only wait on 1MB of codebook, and
            # later chunks stream in behind the PE.
            eT_sbs = []
            for g in range(NG):
                t = epool.tile([P, KC, FD], mm_dt, tag=f"eT{g}")
                for k in range(KC):
                    nc.sync.dma_start(
                        t[:, k, :],
                        eT[k * P:(k + 1) * P, g * FD:(g + 1) * FD],
                    )
                eT_sbs.append(t)

            ST = 2  # token tiles per x super-tile load (256 tokens, 1KB runs)
            xT_sb = None
            for m in range(NT):
                ms = slice(m * P, (m + 1) * P)
                if m % ST == 0:
                    xT_sb = xpool.tile([P, KC, ST * P], mm_dt)
                    ss = slice(m * P, (m + ST) * P)
                    nc.sync.dma_start(
                        xT_sb[:],
                        xT[:, :, ss].rearrange("k p t -> p k t"),
                    )
                xs = slice((m % ST) * P, (m % ST + 1) * P)

                # two half-row dist buffers per tile for finer recycling
                gmax = mpool.tile([P, NGRP], f32)
                for h in range(2):
                    dist_sb = dpool.tile([P, C // 2], f32, tag="dist_h")
                    for gg in range(NG // 2):
                        g = h * (NG // 2) + gg
                        gs = slice(gg * FD, (gg + 1) * FD)
                        ps = pspool.tile([P, FD], f32)
                        for k in range(KC):
                            nc.tensor.matmul(
                                ps[:],
                                xT_sb[:, k, xs],
                                eT_sbs[g][:, k, :],
                                start=(k == 0),
                                stop=(k == KC - 1),
                            )
                        nc.scalar.copy(dist_sb[:, gs], ps[:])
                        # stream out in 512KB chunks of 2 groups
                        if gg % 2 == 1:
                            cs = slice((gg - 1) * FD, (gg + 1) * FD)
                            ds = slice(h * C // 2 + (gg - 1) * FD,
                                       h * C // 2 + (gg + 1) * FD)
                            nc.sync.dma_start(dist[ms, ds], dist_sb[:, cs])
                    nc.vector.tensor_reduce(
                        gmax[:, h * NGRP // 2:(h + 1) * NGRP // 2],
                        dist_sb[:].rearrange("p (g w) -> p g w", w=GW),
                        axis=mybir.AxisListType.X,
                        op=mybir.AluOpType.max,
                    )
                max8 = mpool.tile([P, 8], f32)
                idx8 = mpool.tile([P, 8], u32)
                nc.vector.max(max8[:], gmax[:])
                nc.vector.max_index(idx8[:], max8[:], gmax[:])
                nc.sync.dma_start(vout[ms, :], max8[:])
                nc.sync.dma_start(iout[ms, :], idx8[:])

    nc.compile()
    return nc


def _host_finish(x_flat, e, dist_flat, max8, idx8):
    """Finish the argmax: exact index inside the winning 32-wide sub-group,
    exact global top-2 gap, f64 fix-up of tokens inside the noise band.

    Returns (embed_ind [T] int64, n_fixed)."""
    T = dist_flat.shape[0]
    g1 = idx8[:, 0].astype(np.int64)                       # winning sub-group
    cols = g1[:, None] * GW + np.arange(GW)[None, :]
    slices = dist_flat[np.arange(T)[:, None], cols]        # [T, GW]
    li = np.argmax(slices, axis=1)
    idx = g1 * GW + li
    top1 = slices[np.arange(T), li]
    s2 = np.partition(slices, GW - 2, axis=1)[:, GW - 2]   # 2nd within group
    second = np.maximum(s2, max8[:, 1])                    # exact global 2nd
    flagged = np.nonzero(top1 - second < GAP_THRESH)[0]
    for t in flagged:
        cand = np.nonzero(dist_flat[t] >= top1[t] - 2.0 * GAP_THRESH)[0]
        exact = e[cand].astype(np.float64) @ x_flat[t].astype(np.float64)
        idx[t] = cand[int(np.argmax(exact))]
    return idx, len(flagged)


def run(x, embeddings, trace=False):
    from concourse.bass_utils import run_bass_kernel_spmd

    if "nc" not in _cache:
        _cache["nc"] = _build()
    nc = _cache["nc"]

    x = np.asarray(x, dtype=np.float32)
    e = np.ascontiguousarray(np.asarray(embeddings, dtype=np.float32)[0])  # [C, D]
    eT = np.ascontiguousarray(e.T)  # [D, C]

    in_maps = [
        {
            "xT": np.ascontiguousarray(x[b].T).reshape(KC, P, N),
            "eT": eT,
        }
        for b in range(B)
    ]
    out = run_bass_kernel_spmd(nc, in_maps, list(range(B)), trace=trace)
    res = out.results

    dist = np.stack([res[b]["dist"] for b in range(B)])[None]  # [1,8,N,C]
    embed_ind = np.empty((B, N), dtype=np.int64)
    nfix = 0
    for b in range(B):
        idx, nf = _host_finish(
            x[b], e, dist[0, b], res[b]["vout"], res[b]["iout"]
        )
        embed_ind[b] = idx
        nfix += nf
    run.last_nfix = nfix

    quantize = e[embed_ind]  # [8, N, D]
    return (quantize, embed_ind.astype(np.int32), dist), out


def kernel(x, embeddings):
    (quantize, embed_ind, dist), _ = run(x, embeddings, trace=False)
    return quantize, embed_ind, dist
